# revision 6
# baseline (speedup 1.0000x reference)
"""DTNNStep Bass kernel for Trainium2 (8 NeuronCores, data-parallel over batch).

Computes, per molecule b:
    dist_h = dist @ W_df + b_df              # [N, N, H]
    atom_h = atom @ W_cf + b_cf              # [N, H]
    gated  = dist_h * atom_h[None, :, :]     # broadcast over i
    out    = tanh((gated @ W_fc) * mask)     # mask == 1 in this benchmark
    result = out.sum(axis=1) + atom          # [N, F]

Strategy: everything is computed in a transposed [feature, j] on-chip layout so
the j-reduction is a free-axis reduce.  dist tiles are PE-transposed to put the
d=100 contraction axis on partitions; biases are folded in as K=1 matmuls
against a constant ones row.
"""

import os
import sys

import numpy as np

for _p in ("/opt/trn_rl_repo", os.path.expanduser("~/.axon_site/_ro/trn_rl_repo")):
    if os.path.isdir(_p) and _p not in sys.path:
        sys.path.insert(0, _p)

import concourse.bass as bass
import concourse.tile as tile
from concourse import bacc, mybir
from concourse.bass import ds
from concourse.bass_utils import run_bass_kernel_spmd
from concourse.masks import make_identity

B, N, NF, ND, NH = 16, 128, 64, 100, 64
NCORES = 8
BPC = B // NCORES  # molecules per core

F32 = mybir.dt.float32

G = 4  # i's per compute group (PSUM free dim = G*N = 512)
LG = 16  # i's per dist DMA load


def _emit(tc):
    nc = tc.nc
    dist = nc.dram_tensor("dist", (BPC, N, N, ND), F32, kind="ExternalInput").ap()
    atom = nc.dram_tensor("atom", (BPC, N, NF), F32, kind="ExternalInput").ap()
    w_cf = nc.dram_tensor("w_cf", (NF, NH), F32, kind="ExternalInput").ap()
    w_df = nc.dram_tensor("w_df", (ND, NH), F32, kind="ExternalInput").ap()
    w_fc = nc.dram_tensor("w_fc", (NH, NF), F32, kind="ExternalInput").ap()
    b_cf = nc.dram_tensor("b_cf", (1, NH), F32, kind="ExternalInput").ap()
    b_df = nc.dram_tensor("b_df", (1, NH), F32, kind="ExternalInput").ap()
    out = nc.dram_tensor("out", (BPC, N, NF), F32, kind="ExternalOutput").ap()

    with (
        tc.tile_pool(name="consts", bufs=1) as consts,
        tc.tile_pool(name="loads", bufs=3) as loads,
        tc.tile_pool(name="work", bufs=3) as work,
        tc.tile_pool(name="perb", bufs=2) as perb,
        tc.tile_pool(name="ppool", bufs=2, space="PSUM") as ppool,
        tc.tile_pool(name="psmall", bufs=1, space="PSUM") as psmall,
    ):
        identity = consts.tile([128, 128], F32)
        make_identity(nc, identity)
        ones_row = consts.tile([1, G * N], F32)
        nc.vector.memset(ones_row, 1.0)

        # Warmup: absorb the identity-ready wait on a PE instruction with no
        # other dependencies (transpose-mode matmuls only support one wait).
        warm_ps = psmall.tile([128, 128], F32, tag="warm_ps")
        nc.tensor.transpose(warm_ps, identity, identity)

        w_df_sb = consts.tile([ND, NH], F32)
        nc.sync.dma_start(w_df_sb, w_df)
        w_cf_sb = consts.tile([NF, NH], F32)
        nc.sync.dma_start(w_cf_sb, w_cf)
        w_fc_sb = consts.tile([NH, NF], F32)
        nc.sync.dma_start(w_fc_sb, w_fc)
        bdf_row = consts.tile([1, NH], F32)
        nc.sync.dma_start(bdf_row, b_df)
        bcf_row = consts.tile([1, NH], F32)
        nc.sync.dma_start(bcf_row, b_cf)

        for b in range(BPC):
            # --- per-molecule prep: atom_hT[h, j] = (atom[b] @ W_cf + b_cf)^T
            atom_in = loads.tile([N, NF], F32, tag="atom_in")
            nc.sync.dma_start(atom_in, atom[b])
            atomT_ps = psmall.tile([NF, N], F32, tag="small_ps")
            nc.tensor.transpose(atomT_ps, atom_in, identity)
            atomT = perb.tile([NF, N], F32, tag="atomT")
            nc.vector.tensor_copy(atomT, atomT_ps)
            ah_ps = psmall.tile([NH, N], F32, tag="small_ps")
            nc.tensor.matmul(ah_ps, w_cf_sb, atomT, start=True, stop=False)
            nc.tensor.matmul(ah_ps, bcf_row, ones_row[:, :N], start=False, stop=True)
            atom_hT = perb.tile([NH, N], F32, tag="atom_hT")
            nc.vector.tensor_copy(atom_hT, ah_ps)

            res_b = perb.tile([NF, N], F32, tag="res_b")  # [f, i] sums over j

            for L in range(N // LG):
                dist_in = loads.tile([N, LG, ND], F32, tag="dist_in")
                nc.sync.dma_start(
                    dist_in, dist[b, ds(L * LG, LG)].rearrange("i j d -> j i d")
                )
                for gq in range(LG // G):
                    g = L * (LG // G) + gq
                    # transpose G dist tiles: [N j, ND d] -> [ND, N] each
                    tp_ps = ppool.tile([ND, G * N], F32, tag="tp")
                    for q in range(G):
                        nc.tensor.transpose(
                            tp_ps[:, ds(q * N, N)],
                            dist_in[:, gq * G + q, :],
                            identity,
                        )
                    distT = work.tile([ND, G * N], F32, tag="distT")
                    nc.scalar.copy(distT, tp_ps)

                    # mm1: dist_h^T = W_df^T @ distT (+ b_df)
                    out1_ps = ppool.tile([NH, G * N], F32, tag="out1")
                    nc.tensor.matmul(out1_ps, w_df_sb, distT, start=True, stop=False)
                    nc.tensor.matmul(out1_ps, bdf_row, ones_row, start=False, stop=True)

                    # gate with atom_h^T (broadcast over the G i's)
                    gatedT = work.tile([NH, G * N], F32, tag="gatedT")
                    nc.vector.tensor_tensor(
                        gatedT.rearrange("h (i j) -> h i j", i=G),
                        out1_ps.rearrange("h (i j) -> h i j", i=G),
                        atom_hT[:, None, :].to_broadcast((NH, G, N)),
                        mybir.AluOpType.mult,
                    )

                    # mm2: out2^T = W_fc^T @ gatedT
                    out2_ps = ppool.tile([NF, G * N], F32, tag="out2")
                    nc.tensor.matmul(out2_ps, w_fc_sb, gatedT, start=True, stop=True)

                    # tanh then reduce over j (innermost free axis)
                    tanh_sb = work.tile([NF, G * N], F32, tag="tanh_sb")
                    nc.scalar.activation(
                        tanh_sb, out2_ps, mybir.ActivationFunctionType.Tanh
                    )
                    nc.vector.tensor_reduce(
                        res_b[:, ds(g * G, G)],
                        tanh_sb.rearrange("f (i j) -> f i j", i=G),
                        axis=mybir.AxisListType.X,
                        op=mybir.AluOpType.add,
                    )

            # --- finalize molecule: out[b] = res_b^T + atom[b]
            resT_ps = psmall.tile([N, NF], F32, tag="small_ps")
            nc.tensor.transpose(resT_ps, res_b, identity[:NF, :NF])
            atom_nat = loads.tile([N, NF], F32, tag="atom_nat")
            nc.sync.dma_start(atom_nat, atom[b])
            out_sb = work.tile([N, NF], F32, tag="out_sb")
            nc.vector.tensor_add(out_sb, resT_ps, atom_nat)
            nc.sync.dma_start(out[b], out_sb)


_NC_CACHE = None


def _get_nc():
    global _NC_CACHE
    if _NC_CACHE is None:
        nc = bacc.Bacc("TRN2", target_bir_lowering=False, debug=False)
        with tile.TileContext(nc) as tc:
            _emit(tc)
        nc.compile()
        _NC_CACHE = nc
    return _NC_CACHE


def _numpy_reference(atom, dist, mask, w_cf, w_df, w_fc, b_cf, b_df):
    dist_h = np.einsum("bijd,dh->bijh", dist, w_df) + b_df
    atom_h = np.einsum("bjf,fh->bjh", atom, w_cf) + b_cf
    gated = dist_h * atom_h[:, None, :, :]
    o = np.einsum("bijh,hf->bijf", gated, w_fc)
    o = np.tanh(o * mask[..., None])
    return (o.sum(axis=2) + atom).astype(np.float32)


def run_sharded(inputs, trace=False):
    """Shard over the batch axis, run on 8 cores, gather. Returns (out, results)."""
    atom = np.ascontiguousarray(np.asarray(inputs["atom_features"], np.float32))
    dist = np.ascontiguousarray(np.asarray(inputs["distance_matrix"], np.float32))
    w_cf = np.ascontiguousarray(np.asarray(inputs["W_cf"], np.float32))
    w_df = np.ascontiguousarray(np.asarray(inputs["W_df"], np.float32))
    w_fc = np.ascontiguousarray(np.asarray(inputs["W_fc"], np.float32))
    b_cf = np.asarray(inputs["b_cf"], np.float32).reshape(1, NH)
    b_df = np.asarray(inputs["b_df"], np.float32).reshape(1, NH)

    nc = _get_nc()
    in_maps = []
    for c in range(NCORES):
        sl = slice(c * BPC, (c + 1) * BPC)
        in_maps.append(
            {
                "dist": dist[sl],
                "atom": atom[sl],
                "w_cf": w_cf,
                "w_df": w_df,
                "w_fc": w_fc,
                "b_cf": b_cf,
                "b_df": b_df,
            }
        )
    res = run_bass_kernel_spmd(nc, in_maps, core_ids=list(range(NCORES)), trace=trace)
    out = np.concatenate([res.results[c]["out"] for c in range(NCORES)], axis=0)
    return out, res


def kernel(**inputs) -> np.ndarray:
    mask = np.asarray(inputs["distance_matrix_mask"], np.float32)
    if not np.all(mask == 1.0):
        # The hardware pipeline folds the (always-ones) mask away; keep a
        # correct path for arbitrary masks.
        return _numpy_reference(
            np.asarray(inputs["atom_features"], np.float32),
            np.asarray(inputs["distance_matrix"], np.float32),
            mask,
            np.asarray(inputs["W_cf"], np.float32),
            np.asarray(inputs["W_df"], np.float32),
            np.asarray(inputs["W_fc"], np.float32),
            np.asarray(inputs["b_cf"], np.float32),
            np.asarray(inputs["b_df"], np.float32),
        )
    out, _ = run_sharded(inputs)
    return out


# revision 7
# speedup vs baseline: 2.3569x; 2.3569x over previous
"""DTNNStep Bass kernel for Trainium2 (8 NeuronCores, data-parallel over batch).

Computes, per molecule b:
    dist_h = dist @ W_df + b_df              # [N, N, H]
    atom_h = atom @ W_cf + b_cf              # [N, H]
    gated  = dist_h * atom_h[None, :, :]     # broadcast over i
    out    = tanh((gated @ W_fc) * mask)     # mask == 1 in this benchmark
    result = out.sum(axis=1) + atom          # [N, F]

Strategy: everything is computed in a transposed [feature, j] on-chip layout so
the j-reduction is a free-axis reduce.  dist tiles are PE-transposed to put the
d=100 contraction axis on partitions; biases are folded in as K=1 matmuls
against a constant ones row.  The PE pipeline (transpose, mm1, mm2) runs in
bf16 (fp32 matmuls are two-pass on trn2); PSUM accumulation stays fp32.
"""

import os
import sys

import numpy as np

for _p in ("/opt/trn_rl_repo", os.path.expanduser("~/.axon_site/_ro/trn_rl_repo")):
    if os.path.isdir(_p) and _p not in sys.path:
        sys.path.insert(0, _p)

import concourse.bass as bass
import concourse.tile as tile
from concourse import bacc, mybir
from concourse.bass import ds
from concourse.bass_utils import run_bass_kernel_spmd
from concourse.masks import make_identity

B, N, NF, ND, NH = 16, 128, 64, 100, 64
NCORES = 8
BPC = B // NCORES  # molecules per core

F32 = mybir.dt.float32
BF16 = mybir.dt.bfloat16

G = 4  # i's per compute group (PSUM free dim = G*N = 512)
LG = 16  # i's per dist DMA load


def _emit(tc):
    nc = tc.nc
    dist = nc.dram_tensor("dist", (BPC, N, N, ND), F32, kind="ExternalInput").ap()
    atom = nc.dram_tensor("atom", (BPC, N, NF), F32, kind="ExternalInput").ap()
    w_cf = nc.dram_tensor("w_cf", (NF, NH), F32, kind="ExternalInput").ap()
    w_df = nc.dram_tensor("w_df", (ND, NH), F32, kind="ExternalInput").ap()
    w_fc = nc.dram_tensor("w_fc", (NH, NF), F32, kind="ExternalInput").ap()
    b_cf = nc.dram_tensor("b_cf", (1, NH), F32, kind="ExternalInput").ap()
    b_df = nc.dram_tensor("b_df", (1, NH), F32, kind="ExternalInput").ap()
    out = nc.dram_tensor("out", (BPC, N, NF), F32, kind="ExternalOutput").ap()

    with (
        tc.tile_pool(name="consts", bufs=1) as consts,
        tc.tile_pool(name="loads", bufs=3) as loads,
        tc.tile_pool(name="work", bufs=3) as work,
        tc.tile_pool(name="perb", bufs=2) as perb,
        tc.tile_pool(name="ppool", bufs=2, space="PSUM") as ppool,
        tc.tile_pool(name="psmall", bufs=1, space="PSUM") as psmall,
    ):
        identity = consts.tile([128, 128], F32)
        make_identity(nc, identity)
        identity_bf = consts.tile([128, 128], BF16)
        make_identity(nc, identity_bf)
        ones_bf = consts.tile([1, G * N], BF16)
        nc.vector.memset(ones_bf, 1.0)
        ones_f32 = consts.tile([1, N], F32)
        nc.vector.memset(ones_f32, 1.0)

        # Warmup: absorb the identity-ready wait on a PE instruction with no
        # other dependencies (transpose-mode matmuls only support one wait).
        warm_ps = psmall.tile([128, 128], F32, tag="warm_ps")
        nc.tensor.transpose(warm_ps, identity, identity)

        # fp32 staging + bf16 casts for the PE-side constants
        w_df_f = consts.tile([ND, NH], F32)
        nc.sync.dma_start(w_df_f, w_df)
        w_df_bf = consts.tile([ND, NH], BF16)
        nc.vector.tensor_copy(w_df_bf, w_df_f)
        w_fc_f = consts.tile([NH, NF], F32)
        nc.sync.dma_start(w_fc_f, w_fc)
        w_fc_bf = consts.tile([NH, NF], BF16)
        nc.vector.tensor_copy(w_fc_bf, w_fc_f)
        bdf_f = consts.tile([1, NH], F32)
        nc.sync.dma_start(bdf_f, b_df)
        bdf_bf = consts.tile([1, NH], BF16)
        nc.vector.tensor_copy(bdf_bf, bdf_f)

        w_cf_sb = consts.tile([NF, NH], F32)
        nc.sync.dma_start(w_cf_sb, w_cf)
        bcf_row = consts.tile([1, NH], F32)
        nc.sync.dma_start(bcf_row, b_cf)

        for b in range(BPC):
            # --- per-molecule prep: atom_hT[h, j] = (atom[b] @ W_cf + b_cf)^T
            atom_in = loads.tile([N, NF], F32, tag="atom_in")
            nc.sync.dma_start(atom_in, atom[b])
            atomT_ps = psmall.tile([NF, N], F32, tag="small_ps")
            nc.tensor.transpose(atomT_ps, atom_in, identity)
            atomT = perb.tile([NF, N], F32, tag="atomT")
            nc.vector.tensor_copy(atomT, atomT_ps)
            ah_ps = psmall.tile([NH, N], F32, tag="small_ps")
            nc.tensor.matmul(ah_ps, w_cf_sb, atomT, start=True, stop=False)
            nc.tensor.matmul(ah_ps, bcf_row, ones_f32, start=False, stop=True)
            atom_hT = perb.tile([NH, N], F32, tag="atom_hT")
            nc.vector.tensor_copy(atom_hT, ah_ps)

            res_b = perb.tile([NF, N], F32, tag="res_b")  # [f, i] sums over j

            for L in range(N // LG):
                dist_in = loads.tile([N, LG, ND], F32, tag="dist_in")
                nc.sync.dma_start(
                    dist_in, dist[b, ds(L * LG, LG)].rearrange("i j d -> j i d")
                )
                dist_bf = loads.tile([N, LG, ND], BF16, tag="dist_bf")
                nc.gpsimd.tensor_copy(dist_bf, dist_in)
                for gq in range(LG // G):
                    g = L * (LG // G) + gq
                    # transpose G dist tiles: [N j, ND d] -> [ND, N] each
                    tp_ps = ppool.tile([ND, G * N], BF16, tag="tp")
                    for q in range(G):
                        nc.tensor.transpose(
                            tp_ps[:, ds(q * N, N)],
                            dist_bf[:, gq * G + q, :],
                            identity_bf,
                        )
                    distT = work.tile([ND, G * N], BF16, tag="distT")
                    nc.scalar.copy(distT, tp_ps)

                    # mm1: dist_h^T = W_df^T @ distT (+ b_df)
                    out1_ps = ppool.tile([NH, G * N], F32, tag="out1")
                    nc.tensor.matmul(out1_ps, w_df_bf, distT, start=True, stop=False)
                    nc.tensor.matmul(out1_ps, bdf_bf, ones_bf, start=False, stop=True)

                    # gate with atom_h^T (broadcast over the G i's)
                    gatedT = work.tile([NH, G * N], BF16, tag="gatedT")
                    nc.vector.tensor_tensor(
                        gatedT.rearrange("h (i j) -> h i j", i=G),
                        out1_ps.rearrange("h (i j) -> h i j", i=G),
                        atom_hT[:, None, :].to_broadcast((NH, G, N)),
                        mybir.AluOpType.mult,
                    )

                    # mm2: out2^T = W_fc^T @ gatedT
                    out2_ps = ppool.tile([NF, G * N], F32, tag="out2")
                    nc.tensor.matmul(out2_ps, w_fc_bf, gatedT, start=True, stop=True)

                    # tanh then reduce over j (innermost free axis)
                    tanh_sb = work.tile([NF, G * N], F32, tag="tanh_sb")
                    nc.scalar.activation(
                        tanh_sb, out2_ps, mybir.ActivationFunctionType.Tanh
                    )
                    nc.vector.tensor_reduce(
                        res_b[:, ds(g * G, G)],
                        tanh_sb.rearrange("f (i j) -> f i j", i=G),
                        axis=mybir.AxisListType.X,
                        op=mybir.AluOpType.add,
                    )

            # --- finalize molecule: out[b] = res_b^T + atom[b]
            resT_ps = psmall.tile([N, NF], F32, tag="small_ps")
            nc.tensor.transpose(resT_ps, res_b, identity[:NF, :NF])
            atom_nat = loads.tile([N, NF], F32, tag="atom_nat")
            nc.sync.dma_start(atom_nat, atom[b])
            out_sb = work.tile([N, NF], F32, tag="out_sb")
            nc.vector.tensor_add(out_sb, resT_ps, atom_nat)
            nc.sync.dma_start(out[b], out_sb)


_NC_CACHE = None


def _get_nc():
    global _NC_CACHE
    if _NC_CACHE is None:
        nc = bacc.Bacc("TRN2", target_bir_lowering=False, debug=False)
        with tile.TileContext(nc) as tc:
            _emit(tc)
        nc.compile()
        _NC_CACHE = nc
    return _NC_CACHE


def _numpy_reference(atom, dist, mask, w_cf, w_df, w_fc, b_cf, b_df):
    dist_h = np.einsum("bijd,dh->bijh", dist, w_df) + b_df
    atom_h = np.einsum("bjf,fh->bjh", atom, w_cf) + b_cf
    gated = dist_h * atom_h[:, None, :, :]
    o = np.einsum("bijh,hf->bijf", gated, w_fc)
    o = np.tanh(o * mask[..., None])
    return (o.sum(axis=2) + atom).astype(np.float32)


def run_sharded(inputs, trace=False):
    """Shard over the batch axis, run on 8 cores, gather. Returns (out, results)."""
    atom = np.ascontiguousarray(np.asarray(inputs["atom_features"], np.float32))
    dist = np.ascontiguousarray(np.asarray(inputs["distance_matrix"], np.float32))
    w_cf = np.ascontiguousarray(np.asarray(inputs["W_cf"], np.float32))
    w_df = np.ascontiguousarray(np.asarray(inputs["W_df"], np.float32))
    w_fc = np.ascontiguousarray(np.asarray(inputs["W_fc"], np.float32))
    b_cf = np.asarray(inputs["b_cf"], np.float32).reshape(1, NH)
    b_df = np.asarray(inputs["b_df"], np.float32).reshape(1, NH)

    nc = _get_nc()
    in_maps = []
    for c in range(NCORES):
        sl = slice(c * BPC, (c + 1) * BPC)
        in_maps.append(
            {
                "dist": dist[sl],
                "atom": atom[sl],
                "w_cf": w_cf,
                "w_df": w_df,
                "w_fc": w_fc,
                "b_cf": b_cf,
                "b_df": b_df,
            }
        )
    res = run_bass_kernel_spmd(nc, in_maps, core_ids=list(range(NCORES)), trace=trace)
    out = np.concatenate([res.results[c]["out"] for c in range(NCORES)], axis=0)
    return out, res


def kernel(**inputs) -> np.ndarray:
    mask = np.asarray(inputs["distance_matrix_mask"], np.float32)
    if not np.all(mask == 1.0):
        # The hardware pipeline folds the (always-ones) mask away; keep a
        # correct path for arbitrary masks.
        return _numpy_reference(
            np.asarray(inputs["atom_features"], np.float32),
            np.asarray(inputs["distance_matrix"], np.float32),
            mask,
            np.asarray(inputs["W_cf"], np.float32),
            np.asarray(inputs["W_df"], np.float32),
            np.asarray(inputs["W_fc"], np.float32),
            np.asarray(inputs["b_cf"], np.float32),
            np.asarray(inputs["b_df"], np.float32),
        )
    out, _ = run_sharded(inputs)
    return out


# revision 11
# speedup vs baseline: 2.9695x; 1.2599x over previous
"""DTNNStep Bass kernel for Trainium2 (8 NeuronCores, data-parallel over batch).

Computes, per molecule b:
    dist_h = dist @ W_df + b_df              # [N, N, H]
    atom_h = atom @ W_cf + b_cf              # [N, H]
    gated  = dist_h * atom_h[None, :, :]     # broadcast over i
    out    = tanh((gated @ W_fc) * mask)     # mask == 1 in this benchmark
    result = out.sum(axis=1) + atom          # [N, F]

Strategy: everything is computed in a transposed [feature, j] on-chip layout so
the j-reduction is a free-axis reduce.  dist tiles are PE-transposed to put the
d=100 contraction axis on partitions; biases are folded in as K=1 matmuls
against a constant ones row.  The PE pipeline (transpose, mm1, mm2) runs in
bf16 (fp32 matmuls are two-pass on trn2); PSUM accumulation stays fp32.
"""

import os
import sys

import numpy as np

for _p in ("/opt/trn_rl_repo", os.path.expanduser("~/.axon_site/_ro/trn_rl_repo")):
    if os.path.isdir(_p) and _p not in sys.path:
        sys.path.insert(0, _p)

import concourse.bass as bass
import concourse.tile as tile
from concourse import bacc, mybir
from concourse.bass import ds
from concourse.bass_utils import run_bass_kernel_spmd
from concourse.masks import make_identity

B, N, NF, ND, NH = 16, 128, 64, 100, 64
NCORES = 8
BPC = B // NCORES  # molecules per core

F32 = mybir.dt.float32
BF16 = mybir.dt.bfloat16

G = 4  # i's per compute group (PSUM free dim = G*N = 512)
LG = 16  # i's per dist DMA load


def _emit(tc):
    nc = tc.nc
    dist = nc.dram_tensor("dist", (BPC, N, N, ND), F32, kind="ExternalInput").ap()
    atom = nc.dram_tensor("atom", (BPC, N, NF), F32, kind="ExternalInput").ap()
    w_cf = nc.dram_tensor("w_cf", (NF, NH), F32, kind="ExternalInput").ap()
    w_df = nc.dram_tensor("w_df", (ND, NH), F32, kind="ExternalInput").ap()
    w_fc = nc.dram_tensor("w_fc", (NH, NF), F32, kind="ExternalInput").ap()
    b_cf = nc.dram_tensor("b_cf", (1, NH), F32, kind="ExternalInput").ap()
    b_df = nc.dram_tensor("b_df", (1, NH), F32, kind="ExternalInput").ap()
    out = nc.dram_tensor("out", (BPC, N, NF), F32, kind="ExternalOutput").ap()

    with (
        tc.tile_pool(name="consts", bufs=1) as consts,
        tc.tile_pool(name="loads", bufs=3) as loads,
        tc.tile_pool(name="work", bufs=3) as work,
        tc.tile_pool(name="perb", bufs=2) as perb,
        tc.tile_pool(name="ppool", bufs=2, space="PSUM") as ppool,
        tc.tile_pool(name="psmall", bufs=1, space="PSUM") as psmall,
    ):
        identity = consts.tile([128, 128], F32)
        make_identity(nc, identity)
        identity_bf = consts.tile([128, 128], BF16)
        make_identity(nc, identity_bf)
        ones_f32 = consts.tile([1, N], F32)
        nc.vector.memset(ones_f32, 1.0)

        # Warmup: absorb the identity-ready wait on a PE instruction with no
        # other dependencies (transpose-mode matmuls only support one wait).
        warm_ps = psmall.tile([128, 128], F32, tag="warm_ps")
        nc.tensor.transpose(warm_ps, identity, identity)

        # fp32 staging + bf16 casts for the PE-side constants.  W_df is
        # augmented with b_df as row ND so mm1 (K=ND+1 against a ones row in
        # distT_aug) folds the bias in for free.
        w_df_aug_f = consts.tile([ND + 1, NH], F32)
        nc.sync.dma_start(w_df_aug_f[:ND], w_df)
        nc.sync.dma_start(w_df_aug_f[ND : ND + 1], b_df)
        w_df_aug = consts.tile([ND + 1, NH], BF16)
        nc.vector.tensor_copy(w_df_aug, w_df_aug_f)
        w_fc_f = consts.tile([NH, NF], F32)
        nc.sync.dma_start(w_fc_f, w_fc)
        w_fc_bf = consts.tile([NH, NF], BF16)
        nc.vector.tensor_copy(w_fc_bf, w_fc_f)

        w_cf_sb = consts.tile([NF, NH], F32)
        nc.sync.dma_start(w_cf_sb, w_cf)
        bcf_row = consts.tile([1, NH], F32)
        nc.sync.dma_start(bcf_row, b_cf)

        for b in range(BPC):
            # --- per-molecule prep: atom_hT[h, j] = (atom[b] @ W_cf + b_cf)^T
            atom_in = loads.tile([N, NF], F32, tag="atom_in")
            nc.sync.dma_start(atom_in, atom[b])
            atomT_ps = psmall.tile([NF, N], F32, tag="small_ps")
            nc.tensor.transpose(atomT_ps, atom_in, identity)
            atomT = perb.tile([NF, N], F32, tag="atomT")
            nc.vector.tensor_copy(atomT, atomT_ps)
            ah_ps = psmall.tile([NH, N], F32, tag="small_ps")
            nc.tensor.matmul(ah_ps, w_cf_sb, atomT, start=True, stop=False)
            nc.tensor.matmul(ah_ps, bcf_row, ones_f32, start=False, stop=True)
            atom_hT = perb.tile([NH, N], F32, tag="atom_hT")
            nc.vector.tensor_copy(atom_hT, ah_ps)

            res_b = perb.tile([NF, N], F32, tag="res_b")  # [f, i] sums over j

            for L in range(N // LG):
                # bf16 cast happens on the DMA wire (SWDGE dtype cast); the
                # extra column ND is set to 1.0 so the transpose delivers a
                # ones row at partition ND for the bias fold.
                dist_bf = loads.tile([N, LG, ND + 1], BF16, tag="dist_bf")
                nc.gpsimd.memset(dist_bf[:, :, ND], 1.0)
                nc.gpsimd.dma_start(
                    dist_bf[:, :, :ND],
                    dist[b, ds(L * LG, LG)].rearrange("i j d -> j i d"),
                )
                for gq in range(LG // G):
                    g = L * (LG // G) + gq
                    # transpose G dist tiles: [N j, ND+1 d] -> [ND+1, N] each
                    tp_ps = ppool.tile([ND + 1, G * N], BF16, tag="tp")
                    for q in range(G):
                        nc.tensor.transpose(
                            tp_ps[:, ds(q * N, N)],
                            dist_bf[:, gq * G + q, :],
                            identity_bf,
                        )
                    distT = work.tile([ND + 1, G * N], BF16, tag="distT")
                    nc.scalar.copy(distT, tp_ps)

                    # mm1: dist_h^T = W_df_aug^T @ distT_aug (bias folded in)
                    out1_ps = ppool.tile([NH, G * N], F32, tag="out1")
                    nc.tensor.matmul(out1_ps, w_df_aug, distT, start=True, stop=True)

                    # gate with atom_h^T (broadcast over the G i's)
                    gatedT = work.tile([NH, G * N], BF16, tag="gatedT")
                    nc.vector.tensor_tensor(
                        gatedT.rearrange("h (i j) -> h i j", i=G),
                        out1_ps.rearrange("h (i j) -> h i j", i=G),
                        atom_hT[:, None, :].to_broadcast((NH, G, N)),
                        mybir.AluOpType.mult,
                    )

                    # mm2: out2^T = W_fc^T @ gatedT
                    out2_ps = ppool.tile([NF, G * N], F32, tag="out2")
                    nc.tensor.matmul(out2_ps, w_fc_bf, gatedT, start=True, stop=True)

                    # tanh then reduce over j (innermost free axis)
                    tanh_sb = work.tile([NF, G * N], F32, tag="tanh_sb")
                    nc.scalar.activation(
                        tanh_sb, out2_ps, mybir.ActivationFunctionType.Tanh
                    )
                    nc.vector.tensor_reduce(
                        res_b[:, ds(g * G, G)],
                        tanh_sb.rearrange("f (i j) -> f i j", i=G),
                        axis=mybir.AxisListType.X,
                        op=mybir.AluOpType.add,
                    )

            # --- finalize molecule: out[b] = res_b^T + atom[b]
            resT_ps = psmall.tile([N, NF], F32, tag="small_ps")
            nc.tensor.transpose(resT_ps, res_b, identity[:NF, :NF])
            atom_nat = loads.tile([N, NF], F32, tag="atom_nat")
            nc.sync.dma_start(atom_nat, atom[b])
            out_sb = work.tile([N, NF], F32, tag="out_sb")
            nc.vector.tensor_add(out_sb, resT_ps, atom_nat)
            nc.sync.dma_start(out[b], out_sb)


_NC_CACHE = None


def _get_nc():
    global _NC_CACHE
    if _NC_CACHE is None:
        nc = bacc.Bacc("TRN2", target_bir_lowering=False, debug=False)
        with tile.TileContext(nc) as tc:
            _emit(tc)
        nc.compile()
        _NC_CACHE = nc
    return _NC_CACHE


def _numpy_reference(atom, dist, mask, w_cf, w_df, w_fc, b_cf, b_df):
    dist_h = np.einsum("bijd,dh->bijh", dist, w_df) + b_df
    atom_h = np.einsum("bjf,fh->bjh", atom, w_cf) + b_cf
    gated = dist_h * atom_h[:, None, :, :]
    o = np.einsum("bijh,hf->bijf", gated, w_fc)
    o = np.tanh(o * mask[..., None])
    return (o.sum(axis=2) + atom).astype(np.float32)


def run_sharded(inputs, trace=False):
    """Shard over the batch axis, run on 8 cores, gather. Returns (out, results)."""
    atom = np.ascontiguousarray(np.asarray(inputs["atom_features"], np.float32))
    dist = np.ascontiguousarray(np.asarray(inputs["distance_matrix"], np.float32))
    w_cf = np.ascontiguousarray(np.asarray(inputs["W_cf"], np.float32))
    w_df = np.ascontiguousarray(np.asarray(inputs["W_df"], np.float32))
    w_fc = np.ascontiguousarray(np.asarray(inputs["W_fc"], np.float32))
    b_cf = np.asarray(inputs["b_cf"], np.float32).reshape(1, NH)
    b_df = np.asarray(inputs["b_df"], np.float32).reshape(1, NH)

    nc = _get_nc()
    in_maps = []
    for c in range(NCORES):
        sl = slice(c * BPC, (c + 1) * BPC)
        in_maps.append(
            {
                "dist": dist[sl],
                "atom": atom[sl],
                "w_cf": w_cf,
                "w_df": w_df,
                "w_fc": w_fc,
                "b_cf": b_cf,
                "b_df": b_df,
            }
        )
    res = run_bass_kernel_spmd(nc, in_maps, core_ids=list(range(NCORES)), trace=trace)
    out = np.concatenate([res.results[c]["out"] for c in range(NCORES)], axis=0)
    return out, res


def kernel(**inputs) -> np.ndarray:
    mask = np.asarray(inputs["distance_matrix_mask"], np.float32)
    if not np.all(mask == 1.0):
        # The hardware pipeline folds the (always-ones) mask away; keep a
        # correct path for arbitrary masks.
        return _numpy_reference(
            np.asarray(inputs["atom_features"], np.float32),
            np.asarray(inputs["distance_matrix"], np.float32),
            mask,
            np.asarray(inputs["W_cf"], np.float32),
            np.asarray(inputs["W_df"], np.float32),
            np.asarray(inputs["W_fc"], np.float32),
            np.asarray(inputs["b_cf"], np.float32),
            np.asarray(inputs["b_df"], np.float32),
        )
    out, _ = run_sharded(inputs)
    return out


# revision 22
# speedup vs baseline: 4.9117x; 1.6541x over previous
"""DTNNStep Bass kernel for Trainium2 (8 NeuronCores, data-parallel over batch).

Computes, per molecule b:
    dist_h = dist @ W_df + b_df              # [N, N, H]
    atom_h = atom @ W_cf + b_cf              # [N, H]
    gated  = dist_h * atom_h[None, :, :]     # broadcast over i
    out    = tanh((gated @ W_fc) * mask)     # mask == 1 in this benchmark
    result = out.sum(axis=1) + atom          # [N, F]

Strategy: everything is computed in a transposed [feature, j] on-chip layout so
the j-reduction is a free-axis reduce.  dist tiles are PE-transposed to put the
d=100 contraction axis on partitions; biases are folded in as K=1 matmuls
against a constant ones row.  The PE pipeline (transpose, mm1, mm2) runs in
bf16 (fp32 matmuls are two-pass on trn2); PSUM accumulation stays fp32.
"""

import os
import sys

import numpy as np

for _p in ("/opt/trn_rl_repo", os.path.expanduser("~/.axon_site/_ro/trn_rl_repo")):
    if os.path.isdir(_p) and _p not in sys.path:
        sys.path.insert(0, _p)

import concourse.bass as bass
import concourse.tile as tile
from concourse import bacc, mybir
from concourse.bass import ds
from concourse.bass_utils import run_bass_kernel_spmd
from concourse.masks import make_identity

B, N, NF, ND, NH = 16, 128, 64, 100, 64
NCORES = 8
BPC = B // NCORES  # molecules per core

F32 = mybir.dt.float32
BF16 = mybir.dt.bfloat16

G = 4  # i's per compute group (PSUM free dim = G*N = 512)
LG = 16  # i's per dist DMA load


def _emit(tc):
    nc = tc.nc
    dist = nc.dram_tensor("dist", (BPC, N, N, ND), F32, kind="ExternalInput").ap()
    atom = nc.dram_tensor("atom", (BPC, N, NF), F32, kind="ExternalInput").ap()
    w_cf = nc.dram_tensor("w_cf", (NF, NH), F32, kind="ExternalInput").ap()
    w_df = nc.dram_tensor("w_df", (ND, NH), F32, kind="ExternalInput").ap()
    w_fc = nc.dram_tensor("w_fc", (NH, NF), F32, kind="ExternalInput").ap()
    b_cf = nc.dram_tensor("b_cf", (1, NH), F32, kind="ExternalInput").ap()
    b_df = nc.dram_tensor("b_df", (1, NH), F32, kind="ExternalInput").ap()
    out = nc.dram_tensor("out", (BPC, N, NF), F32, kind="ExternalOutput").ap()

    with (
        tc.tile_pool(name="consts", bufs=1) as consts,
        tc.tile_pool(name="loads", bufs=3) as loads,
        tc.tile_pool(name="work", bufs=3) as work,
        tc.tile_pool(name="perb", bufs=2) as perb,
        tc.tile_pool(name="ppool", bufs=2, space="PSUM") as ppool,
        tc.tile_pool(name="psmall", bufs=1, space="PSUM") as psmall,
    ):
        identity = consts.tile([128, 128], F32)
        make_identity(nc, identity)
        identity_bf = consts.tile([128, 128], BF16)
        make_identity(nc, identity_bf)
        ones_f32 = consts.tile([1, N], F32)
        nc.vector.memset(ones_f32, 1.0)

        # Warmup: absorb the identity-ready wait on a PE instruction with no
        # other dependencies (transpose-mode matmuls only support one wait).
        warm_ps = psmall.tile([128, 128], F32, tag="warm_ps")
        nc.tensor.transpose(warm_ps, identity, identity)

        # fp32 staging + bf16 casts for the PE-side constants.  W_df is
        # augmented with b_df as row ND so mm1 (K=ND+1 against a ones row in
        # distT_aug) folds the bias in for free.
        w_df_aug_f = consts.tile([ND + 1, NH], F32)
        nc.sync.dma_start(w_df_aug_f[:ND], w_df)
        nc.sync.dma_start(w_df_aug_f[ND : ND + 1], b_df)
        w_df_aug = consts.tile([ND + 1, NH], BF16)
        nc.vector.tensor_copy(w_df_aug, w_df_aug_f)
        # W_fc stacked twice vertically so the partition-hi mm2 has its
        # stationary at the same base partition as its rhs.
        w_fc_f = consts.tile([2 * NH, NF], F32)
        nc.sync.dma_start(w_fc_f[:NH], w_fc)
        nc.sync.dma_start(w_fc_f[NH:], w_fc)
        w_fc_bf = consts.tile([2 * NH, NF], BF16)
        nc.vector.tensor_copy(w_fc_bf, w_fc_f)

        # W_cf (and b_cf) duplicated horizontally so atom_hT comes out
        # stacked twice vertically: [2*NH, N] for the partition-packed gate.
        w_cf_dup = consts.tile([NF, 2 * NH], F32)
        nc.sync.dma_start(w_cf_dup[:, :NH], w_cf)
        nc.sync.dma_start(w_cf_dup[:, NH:], w_cf)
        bcf_dup = consts.tile([1, 2 * NH], F32)
        nc.sync.dma_start(bcf_dup[:, :NH], b_cf)
        nc.sync.dma_start(bcf_dup[:, NH:], b_cf)

        # Partition half u=0 handles rows i in [0, N/2), u=1 handles
        # [N/2, N); within a half, pair-group t slot q maps to i = 4*t + q,
        # so outputs come out in natural row order (no permuted APs needed).
        NT = N // 8  # pair-groups per molecule

        for b in range(BPC):
            # --- per-molecule prep: atom_hT2[(u h), j] = (atom[b]@W_cf+b_cf)^T x2
            atom_in = loads.tile([N, NF], F32, tag="atom_in")
            nc.sync.dma_start(atom_in, atom[b])
            atomT_ps = psmall.tile([NF, N], F32, tag="small_ps")
            nc.tensor.transpose(atomT_ps, atom_in, identity)
            atomT = perb.tile([NF, N], F32, tag="atomT")
            nc.vector.tensor_copy(atomT, atomT_ps)
            ah_ps = psmall.tile([2 * NH, N], F32, tag="small_ps")
            nc.tensor.matmul(ah_ps, w_cf_dup, atomT, start=True, stop=False)
            nc.tensor.matmul(ah_ps, bcf_dup, ones_f32, start=False, stop=True)
            atom_hT2 = perb.tile([2 * NH, N], F32, tag="atom_hT2")
            nc.vector.tensor_copy(atom_hT2, ah_ps)

            # res_pack[(u f), (t q)] accumulates the j-sums per output row.
            # The ACT accumulator read-modify-writes its destination, so zero it.
            res_pack = perb.tile([2 * NF, 4 * NT], F32, tag="res_pack")
            nc.vector.memset(res_pack, 0.0)

            for L in range(N // LG):
                # bf16 cast happens on the DMA wire (SWDGE dtype cast); the
                # extra column ND is set to 1.0 so the transpose delivers a
                # ones row at partition ND for the bias fold.  Halves 0:8 and
                # 8:16 of the i axis come from the two N/2 row blocks.
                dist_bf = loads.tile([N, LG, ND + 1], BF16, tag="dist_bf")
                nc.gpsimd.memset(dist_bf[:, :, ND], 1.0)
                half = LG // 2
                nc.gpsimd.dma_start(
                    dist_bf[:, :half, :ND],
                    dist[b, ds(L * half, half)].rearrange("i j d -> j i d"),
                )
                nc.gpsimd.dma_start(
                    dist_bf[:, half:, :ND],
                    dist[b, ds(N // 2 + L * half, half)].rearrange("i j d -> j i d"),
                )
                for tt in range(LG // 8):
                    t = L * (LG // 8) + tt
                    # transpose 8 dist tiles: [N j, ND+1 d] -> [ND+1, N] each;
                    # q 0..3 from the lo i-block, 4..7 from the hi i-block
                    tp_ps = ppool.tile([ND + 1, 8 * N], BF16, tag="tp")
                    for q in range(4):
                        nc.tensor.transpose(
                            tp_ps[:, ds(q * N, N)],
                            dist_bf[:, tt * 4 + q, :],
                            identity_bf,
                        )
                    for q in range(4):
                        nc.tensor.transpose(
                            tp_ps[:, ds((4 + q) * N, N)],
                            dist_bf[:, half + tt * 4 + q, :],
                            identity_bf,
                        )
                    distT = work.tile([ND + 1, 8 * N], BF16, tag="distT")
                    nc.scalar.copy(distT, tp_ps)

                    # mm1: dist_h^T = W_df_aug^T @ distT_aug (bias folded in);
                    # the two 4-i halves land on partition halves of out1_ps
                    # and run concurrently on separate PE column groups.
                    out1_ps = ppool.tile([2 * NH, G * N], F32, tag="out1")
                    nc.tensor.matmul(
                        out1_ps[:NH], w_df_aug, distT[:, : G * N], start=True, stop=True
                    )
                    nc.tensor.matmul(
                        out1_ps[NH:], w_df_aug, distT[:, G * N :], start=True, stop=True
                    )

                    # gate with atom_h^T (broadcast over the G i's per half)
                    gatedT = work.tile([2 * NH, G * N], BF16, tag="gatedT")
                    nc.vector.tensor_tensor(
                        gatedT.rearrange("h (i j) -> h i j", i=G),
                        out1_ps.rearrange("h (i j) -> h i j", i=G),
                        atom_hT2[:, None, :].to_broadcast((2 * NH, G, N)),
                        mybir.AluOpType.mult,
                    )

                    # mm2: out2^T = W_fc^T @ gatedT, per partition half
                    out2_ps = ppool.tile([2 * NF, G * N], F32, tag="out2")
                    nc.tensor.matmul(
                        out2_ps[:NF], w_fc_bf[:NH], gatedT[:NH], start=True, stop=True
                    )
                    nc.tensor.matmul(
                        out2_ps[NF:], w_fc_bf[NH:], gatedT[NH:], start=True, stop=True
                    )

                    # tanh then reduce over j (innermost free axis)
                    tanh_sb = work.tile([2 * NF, G * N], F32, tag="tanh_sb")
                    nc.scalar.activation(
                        tanh_sb, out2_ps, mybir.ActivationFunctionType.Tanh
                    )
                    nc.vector.tensor_reduce(
                        res_pack[:, ds(4 * t, G)],
                        tanh_sb.rearrange("f (i j) -> f i j", i=G),
                        axis=mybir.AxisListType.X,
                        op=mybir.AluOpType.add,
                    )

            # --- finalize molecule: out[b] = res_pack^T + atom[b] (per half)
            for u in range(2):
                resT_ps = psmall.tile([N // 2, NF], F32, tag="small_ps")
                nc.tensor.transpose(
                    resT_ps,
                    res_pack[ds(u * NF, NF)],
                    identity[ds(u * NF, NF), ds(u * NF, N // 2)],
                )
                atom_nat = loads.tile([N // 2, NF], F32, tag="atom_nat")
                nc.sync.dma_start(atom_nat, atom[b, ds(u * (N // 2), N // 2)])
                out_sb = work.tile([N // 2, NF], F32, tag="out_sb")
                nc.vector.tensor_add(out_sb, resT_ps, atom_nat)
                nc.sync.dma_start(out[b, ds(u * (N // 2), N // 2)], out_sb)


_NC_CACHE = None


def _get_nc():
    global _NC_CACHE
    if _NC_CACHE is None:
        nc = bacc.Bacc("TRN2", target_bir_lowering=False, debug=False)
        with tile.TileContext(nc) as tc:
            _emit(tc)
        nc.compile()
        _NC_CACHE = nc
    return _NC_CACHE


def _numpy_reference(atom, dist, mask, w_cf, w_df, w_fc, b_cf, b_df):
    dist_h = np.einsum("bijd,dh->bijh", dist, w_df) + b_df
    atom_h = np.einsum("bjf,fh->bjh", atom, w_cf) + b_cf
    gated = dist_h * atom_h[:, None, :, :]
    o = np.einsum("bijh,hf->bijf", gated, w_fc)
    o = np.tanh(o * mask[..., None])
    return (o.sum(axis=2) + atom).astype(np.float32)


def run_sharded(inputs, trace=False):
    """Shard over the batch axis, run on 8 cores, gather. Returns (out, results)."""
    atom = np.ascontiguousarray(np.asarray(inputs["atom_features"], np.float32))
    dist = np.ascontiguousarray(np.asarray(inputs["distance_matrix"], np.float32))
    w_cf = np.ascontiguousarray(np.asarray(inputs["W_cf"], np.float32))
    w_df = np.ascontiguousarray(np.asarray(inputs["W_df"], np.float32))
    w_fc = np.ascontiguousarray(np.asarray(inputs["W_fc"], np.float32))
    b_cf = np.asarray(inputs["b_cf"], np.float32).reshape(1, NH)
    b_df = np.asarray(inputs["b_df"], np.float32).reshape(1, NH)

    nc = _get_nc()
    in_maps = []
    for c in range(NCORES):
        sl = slice(c * BPC, (c + 1) * BPC)
        in_maps.append(
            {
                "dist": dist[sl],
                "atom": atom[sl],
                "w_cf": w_cf,
                "w_df": w_df,
                "w_fc": w_fc,
                "b_cf": b_cf,
                "b_df": b_df,
            }
        )
    res = run_bass_kernel_spmd(nc, in_maps, core_ids=list(range(NCORES)), trace=trace)
    out = np.concatenate([res.results[c]["out"] for c in range(NCORES)], axis=0)
    return out, res


def kernel(**inputs) -> np.ndarray:
    mask = np.asarray(inputs["distance_matrix_mask"], np.float32)
    if not np.all(mask == 1.0):
        # The hardware pipeline folds the (always-ones) mask away; keep a
        # correct path for arbitrary masks.
        return _numpy_reference(
            np.asarray(inputs["atom_features"], np.float32),
            np.asarray(inputs["distance_matrix"], np.float32),
            mask,
            np.asarray(inputs["W_cf"], np.float32),
            np.asarray(inputs["W_df"], np.float32),
            np.asarray(inputs["W_fc"], np.float32),
            np.asarray(inputs["b_cf"], np.float32),
            np.asarray(inputs["b_df"], np.float32),
        )
    out, _ = run_sharded(inputs)
    return out
